# revision 34
# baseline (speedup 1.0000x reference)
"""MoE-routed K-cluster autoencoder kernel for 8 Trainium2 NeuronCores.

Strategy
--------
Each row of x is reconstructed by the autoencoder of its kmeans cluster.
Computing all K experts densely for every row (like the reference) does
10x the needed matmul work, so we *route*:

  host:   sort rows by cluster, pack them into fixed-capacity "slots"
          (one cluster per slot; slot capacity R is chosen per label
          histogram to minimize padding), pre-transpose so features lie
          on SBUF partitions (6 full 128-row k-tiles + one 16-row tail,
          so bulk DMAs engage all 16 SDMA engines), and flatten each
          row-chunk k-major so every DMA moves long contiguous runs.
  device: per slot, run the 6-layer MLP chain as feature-major matmuls
          (outT = W.T @ actsT). Weight-stationary groups stream all of a
          slot's row-chunks back-to-back; the next slot's encoder-0 work
          is software-pipelined into the current slot's serial mid-layers
          (the PE is in-order, so independent filler matmuls are emitted
          where the layer chain would stall, and cheap "dummy" matmuls
          are emitted when no filler is left so the HAM clock gate never
          drops the PE to 1.2 GHz); PSUM->SBUF bias+ReLU evictions
          alternate between ScalarE and VectorE; decoder-2 runs
          chunk-major with per-chunk y writeback DMAs so the output
          streams continuously instead of piling up at the end.
  host:   scatter the per-slot outputs back to original row order.

The kernel is simultaneously PE-bound (~63us of matmul streaming) and
DMA-bound (~60us at the ~300 GB/s effective per-core rate), so the
single in-order DMA queue is sequenced just-in-time: slot 0/1 tensors
are split so each consumer's bytes arrive right before its matmuls;
later slots stream as whole-slot transfers.  Weights are packed by
partition-row count (128/64/16-row regions) so no zero rows move.

Two operand-dtype modes (MODE): "bf16" (default -- half the x/weight DMA
bytes, ~5.6e-3 scale-relative absmax err) and "f32r" (fp32 bytes
end-to-end, slower, ~3.4e-4 err).
"""

import numpy as np

import concourse.tile as tile
from concourse import bacc, mybir
from concourse.bass_utils import run_bass_kernel_spmd

N_CORES = 8
B, D, H1, H2, L, K = 32768, 784, 256, 64, 16, 10
KM = 6           # 6 full 128-row k/out-tiles along D (128-row tensors
                 # DMA ~40% faster per engine and engage all 16 engines)
TR = D - 128 * KM  # 16-row tail tile
# The e0 tail k-tile's stationary operand is zero-padded to 128 rows:
# row-group-restricted (<=32-row) stationary operands stall the PE ~2x,
# and zero weight rows make the (finite) garbage in the moving operand's
# padding rows contribute exactly zero.

# per-slot packed weight layout, grouped by partition-row count so DMAs
# move (almost) no zero rows (column offsets in a [128, WSLOT] block):
#   rows 0:128 -- e1, d2, e0 (main k-tiles + zero-padded tail)
#   rows 0:64  -- e2, d1
#   rows 0:16  -- d0
_E1 = 0                  # [128 x 128]   e1 two 64-col k-halves
_D2 = 128                # [128 x 1568]  d2 (k-major, out-tiles 6x128+16)
_E0M = 1696              # [128 x 1536]  e0 k=0..5, m-major pairs of 128
_E0T = 3232              # [128 x 256]   e0 tail (rows 16:128 zero)
_E2 = 3488               # [64 x 16]
_D1 = 3504               # [64 x 256]
_D0 = 3760               # [16 x 64]
WSLOT = 3824
BSLOT = 14       # bias columns per slot: 2 + 1 + 1 + 1 + 2 + 7

# fallback (slots_per_core, rows_per_slot) for f32r / degenerate cases
_CONFIGS = [(4, 1152), (4, 1280), (8, 640), (16, 320), (32, 160)]

_F32 = mybir.dt.float32
_F32R = mybir.dt.float32r
_BF16 = mybir.dt.bfloat16
_RELU = mybir.ActivationFunctionType.Relu

# matmul-operand dtype mode: "bf16" (default, fast) or "f32r" (precise)
MODE = "bf16"


def _mdt_view(ap, mode):
    return ap.bitcast(_F32R) if mode == "f32r" else ap


def _chunks(R, mode="f32r"):
    """Split R rows into moving-operand chunks <=512."""
    if mode == "bf16":
        # 32-aligned chunk sizes: ragged free dims slow every drain's
        # ACT/DVE instruction by ~100ns, which gates decoder-2's stream
        n = max(1, (R + 511) // 512)
        nb, extra = divmod(R // 32, n)
        return [32 * (nb + (1 if i < extra else 0)) for i in range(n)]
    out, rem = [], R
    while rem > 0:
        c = min(512, rem)
        if c == 512 and 0 < rem - c < 256:
            c = max(256, min(512, (rem + 1) // 2))
        out.append(c)
        rem -= c
    return out


def _build_program(S, R, mode):
    mdt = _F32R if mode == "f32r" else _BF16
    idt = _F32 if mode == "f32r" else _BF16
    pipelined = mode == "bf16"
    ncols = S * R
    nc = bacc.Bacc("TRN2", target_bir_lowering=False, debug=False)
    xt = nc.dram_tensor("xt", [128, ncols * KM], idt, kind="ExternalInput").ap()
    xtt = nc.dram_tensor("xtt", [TR, ncols], idt, kind="ExternalInput").ap()
    wp = nc.dram_tensor("wp", [128, S * WSLOT], idt, kind="ExternalInput").ap()
    bp = nc.dram_tensor("bp", [128, S * BSLOT], _F32, kind="ExternalInput").ap()
    yt = nc.dram_tensor("yt", [128, ncols * KM], idt, kind="ExternalOutput").ap()
    ytt = nc.dram_tensor("ytt", [TR, ncols], idt, kind="ExternalOutput").ap()

    chunks = _chunks(R, mode)
    NCH = len(chunks)
    roff = []
    cum = 0
    for nch in chunks:
        roff.append(cum)
        cum += nch
    XS_BUFS = 3 if pipelined else 2
    W_BUFS = 4 if pipelined else 2
    H1_BUFS = 10 if pipelined else 6
    SM_BUFS = 6 if pipelined else 3

    with tile.TileContext(nc) as tc:
        with (
            tc.tile_pool(name="wpool", bufs=1) as wpool,
            tc.tile_pool(name="iopool", bufs=1) as iopool,
            tc.tile_pool(name="apool", bufs=1) as apool,
            tc.tile_pool(name="pspool", bufs=1, space="PSUM") as pspool,
        ):
            bsb = wpool.tile([128, S * BSLOT], _F32, tag="b", name="bsb",
                             bufs=1)
            wu = wpool.tile([128, 512], _BF16, tag="wu", name="wu", bufs=1)
            wups = [pspool.tile([128, 512], _F32, tag="ps", name="wups",
                                bufs=7) for _ in range(4)]
            # dedicated keep-warm PSUM target, outside the "ps" rotation:
            # dummy matmuls only ever WAW-chain on the in-order PE.
            dum = pspool.tile([128, 128], _F32, tag="dum", name="dum",
                              bufs=1)

            def dummy(n):
                """n cheap 128-col matmuls: keeps the PE HAM-warm where
                the schedule would otherwise leave the engine idle."""
                for _ in range(n):
                    nc.tensor.matmul(dum, wu[:, 0:128], wu[:, 0:128],
                                     start=True, stop=True)

            def bias(lo, col):
                return bsb[0:lo, col:col + 1]

            def ps_tile(parts, nch):
                return pspool.tile([parts, nch], _F32, tag="ps", name="ps",
                                   bufs=7)

            drain_i = [0]

            def drain_relu(out, ps, bias_ap):
                """bias+ReLU PSUM->SBUF eviction, alternating ACT/DVE."""
                drain_i[0] += 1
                if drain_i[0] % 2:
                    nc.scalar.activation(out, ps, _RELU, bias=bias_ap)
                else:
                    nc.vector.tensor_scalar(out, ps, bias_ap, 0.0,
                                            mybir.AluOpType.add,
                                            mybir.AluOpType.max)

            def drain_bias(out, ps, bias_ap):
                """bias-only PSUM->SBUF eviction, alternating ACT/DVE."""
                drain_i[0] += 1
                if drain_i[0] % 2:
                    nc.scalar.add(out, ps, bias_ap)
                else:
                    nc.vector.tensor_scalar_add(out, ps, bias_ap)

            def drain_relu_split(out, ps, bias_ap, nch):
                """bias+ReLU eviction split across BOTH engines: ~half the
                latency, for drains the serial mid-layer chain waits on."""
                h = (nch // 2 + 31) // 32 * 32
                nc.scalar.activation(out[:, 0:h], ps[:, 0:h], _RELU,
                                     bias=bias_ap)
                nc.vector.tensor_scalar(out[:, h:nch], ps[:, h:nch],
                                        bias_ap, 0.0,
                                        mybir.AluOpType.add,
                                        mybir.AluOpType.max)

            res = {}

            def dma_w(s, c0, c1, rows=128):
                r = res[s]
                nc.sync.dma_start(
                    out=r["w"][0:rows, c0:c1],
                    in_=_mdt_view(wp[0:rows, s * WSLOT + c0:s * WSLOT + c1],
                                  mode))

            def dma_x(s, e0, e1):
                r = res[s]
                nc.sync.dma_start(
                    out=r["xs"][:, e0:e1],
                    in_=_mdt_view(xt[:, s * R * KM + e0:s * R * KM + e1],
                                  mode))

            # persistent x-tail staging: one [128, 3R] tile, rows 0:16
            # rewritten per slot (s%3 rotation), rows 16:128 zeroed once
            # so the zero-padded tail weights see finite values.
            xpd = wpool.tile([128, 3 * R], mdt, tag="xpd", name="xpd",
                             bufs=1)
            nc.gpsimd.memset(xpd, 0)

            def dma_xt(s):
                nc.sync.dma_start(
                    out=xpd[0:TR, (s % 3) * R:(s % 3) * R + R],
                    in_=_mdt_view(xtt[:, s * R:(s + 1) * R], mode))

            def alloc_slot(s):
                if s in res or s >= S:
                    return
                w = wpool.tile([128, WSLOT], mdt, tag="w", name="w",
                               bufs=W_BUFS)
                xs = iopool.tile([128, R * KM], mdt, tag="xs", name="xs",
                                 bufs=XS_BUFS)
                res[s] = {"w": w, "xs": xs, "bb": s * BSLOT,
                          "h1": [[None] * NCH, [None] * NCH]}

            def ensure_slot(s):
                """Steady-state slot prefetch, grouped by row count."""
                if s in res or s >= S:
                    return
                alloc_slot(s)
                dma_w(s, 0, _E2)                 # 128-row region
                dma_w(s, _E2, WSLOT, rows=64)    # 64/16-row region
                dma_x(s, 0, R * KM)
                dma_xt(s)

            def e0_unit(s, m, ci):
                """One filler unit: chunk ci's full e0 contraction for
                m-half m (7 matmuls, one short-lived PSUM bank)."""
                r = res[s]
                nch = chunks[ci]
                c6 = KM * roff[ci]
                ps = ps_tile(128, nch)
                for k in range(KM):
                    wk = r["w"][:, _E0M + k * 256 + 128 * m:
                                _E0M + k * 256 + 128 * m + 128]
                    nc.tensor.matmul(ps, wk,
                                     r["xs"][:, c6 + k * nch:c6 + (k + 1) * nch],
                                     start=(k == 0), stop=False)
                wk = r["w"][:, _E0T + 128 * m:_E0T + 128 * m + 128]
                nc.tensor.matmul(
                    ps, wk,
                    xpd[:, (s % 3) * R + roff[ci]:(s % 3) * R + roff[ci] + nch],
                    start=False, stop=True)
                t = apool.tile([128, nch], mdt, tag="h1", name="h1",
                               bufs=H1_BUFS)
                drain_relu(t, ps, bias(128, r["bb"] + m))
                r["h1"][m][ci] = t

            E0_ORDER = [(m, ci) for m in range(2) for ci in range(NCH)]

            if pipelined:
                # Startup: the DMA queue is in-order, so sequence slot-0/1
                # pieces just-in-time for the PE.
                alloc_slot(0)
                alloc_slot(1)
                dma_w(0, _E0M, _E2)              # s0 e0 weights
                dma_xt(0)                        # s0 x tail (tiny)
                dma_x(0, 0, KM * roff[1] if NCH > 1 else R * KM)  # chunk 0
                nc.sync.dma_start(out=bsb, in_=bp)
                dma_w(1, _E0M, _E2)              # s1 e0 weights
                for ci in range(1, NCH):         # s0 chunks 1..
                    dma_x(0, KM * roff[ci], KM * (roff[ci] + chunks[ci]))
                dma_w(0, _E1, _D2)               # s0 e1
                dma_w(0, _E2, WSLOT, rows=64)    # s0 e2+d1+d0
                dma_xt(1)
                if NCH > 1:
                    dma_x(1, 0, KM * roff[1])    # s1 chunk 0
                dma_w(0, _D2, _E0M)              # s0 d2
                if NCH > 1:
                    dma_x(1, KM * roff[1], R * KM)
                else:
                    dma_x(1, 0, R * KM)
                dma_w(1, 0, _E0M)                # s1 e1+d2
                dma_w(1, _E2, WSLOT, rows=64)

                # PE pre-warm: open the HAM clock gate while DMAs land.
                nc.vector.memset(wu, 0)
                for i in range(7):
                    nc.tensor.matmul(wups[i % 4], wu[:, 0:128], wu,
                                     start=True, stop=True)
                dummy(10)

                # slot 0 runs chunk-major, self-pacing with the head DMAs
                for ci in range(NCH):
                    for m in range(2):
                        e0_unit(0, m, ci)
                    if ci + 1 < NCH:
                        dummy(6)
            for s in range(S):
                if pipelined:
                    ensure_slot(s + 2)
                    filler = iter(E0_ORDER) if s + 1 < S else iter([])
                else:
                    if s == 0:
                        nc.sync.dma_start(out=bsb, in_=bp)
                    ensure_slot(s)
                    for m, ci in E0_ORDER:
                        e0_unit(s, m, ci)
                    filler = iter([])

                def fill(n, pad=True):
                    for _ in range(n):
                        mk = next(filler, None)
                        if mk is not None:
                            e0_unit(s + 1, *mk)
                        elif pad and pipelined:
                            dummy(6 if s == S - 1 else 3)

                r = res[s]
                w, bb, h1 = r["w"], r["bb"], r["h1"]

                # encoder 1: [256 -> 64]
                ps = [None] * NCH
                for k in range(2):
                    wk = w[0:128, _E1 + 64 * k:_E1 + 64 * k + 64]
                    for ci, nch in enumerate(chunks):
                        if k == 0:
                            ps[ci] = ps_tile(64, nch)
                        nc.tensor.matmul(ps[ci], wk, h1[k][ci],
                                         start=(k == 0), stop=(k == 1))
                h2 = []
                for ci, nch in enumerate(chunks):
                    t = apool.tile([64, nch], mdt, tag="h2", name="h2", bufs=SM_BUFS)
                    drain_relu_split(t, ps[ci], bias(64, bb + 2), nch)
                    h2.append(t)
                fill(1)
                if s == S - 1:
                    dummy(4)

                # encoder 2: [64 -> 16]
                ps = [None] * NCH
                wk = w[0:64, _E2:_E2 + 16]
                for ci, nch in enumerate(chunks):
                    ps[ci] = ps_tile(16, nch)
                    nc.tensor.matmul(ps[ci], wk, h2[ci], start=True, stop=True)
                z = []
                for ci, nch in enumerate(chunks):
                    t = apool.tile([16, nch], mdt, tag="z", name="z", bufs=SM_BUFS)
                    drain_relu_split(t, ps[ci], bias(16, bb + 3), nch)
                    z.append(t)
                fill(1)
                if s == S - 1:
                    dummy(4)

                # decoder 0: [16 -> 64]
                ps = [None] * NCH
                wk = w[0:16, _D0:_D0 + 64]
                for ci, nch in enumerate(chunks):
                    ps[ci] = ps_tile(64, nch)
                    nc.tensor.matmul(ps[ci], wk, z[ci], start=True, stop=True)
                a1 = []
                for ci, nch in enumerate(chunks):
                    t = apool.tile([64, nch], mdt, tag="a1", name="a1", bufs=SM_BUFS)
                    drain_relu_split(t, ps[ci], bias(64, bb + 4), nch)
                    a1.append(t)
                fill(1)
                if s == S - 1:
                    dummy(4)

                # decoder 1: [64 -> 256]
                a2 = [[None] * NCH, [None] * NCH]
                for m in range(2):
                    wk = w[0:64, _D1 + 128 * m:_D1 + 128 * m + 128]
                    ps = [None] * NCH
                    for ci, nch in enumerate(chunks):
                        ps[ci] = ps_tile(128, nch)
                        nc.tensor.matmul(ps[ci], wk, a1[ci],
                                         start=True, stop=True)
                    for ci, nch in enumerate(chunks):
                        t = apool.tile([128, nch], mdt, tag="a2", name="a2",
                                       bufs=7)
                        drain_relu_split(t, ps[ci], bias(128, bb + 5 + m),
                                         nch)
                        a2[m][ci] = t
                    fill(1)

                # decoder 2: [256 -> 784], bias only.  Chunk-major with a
                # per-chunk y writeback DMA; out-tiles are 6x128 + 16.
                ys = iopool.tile([128, R * KM], idt, tag="ys", name="ys",
                                 bufs=2)
                yst = iopool.tile([TR, R], idt, tag="yst", name="yst",
                                  bufs=2)
                for ci, nch in enumerate(chunks):
                    c6 = KM * roff[ci]
                    split = (s == S - 1 and ci == NCH - 1)
                    # last chunk of the last slot computes its tail
                    # out-tile FIRST so the whole-slot ytt flush issues
                    # before the final main-tile y pieces.
                    order = [KM] + list(range(KM)) if split \
                        else list(range(KM + 1))
                    for mi, mm in enumerate(order):
                        outw = 128 if mm < KM else TR
                        ps = ps_tile(outw, nch)
                        for k in range(2):
                            co = _D2 + 784 * k + (mm * 128 if mm < KM else 768)
                            nc.tensor.matmul(ps, w[0:128, co:co + outw],
                                             a2[k][ci],
                                             start=(k == 0), stop=(k == 1))
                        if mm < KM:
                            drain_bias(ys[:, c6 + mm * nch:c6 + (mm + 1) * nch],
                                       ps, bias(128, bb + 7 + mm))
                        else:
                            drain_bias(yst[:, roff[ci]:roff[ci] + nch],
                                       ps, bias(TR, bb + 13))
                            if split:
                                nc.sync.dma_start(
                                    out=ytt[:, s * R:(s + 1) * R], in_=yst)
                        if mi == 4 and not split and ci < 2:
                            fill(1)
                        if mi == 3 and split:
                            nc.sync.dma_start(
                                out=yt[:, s * R * KM + c6:
                                       s * R * KM + c6 + 3 * nch],
                                in_=ys[:, c6:c6 + 3 * nch])
                        if mi == 5 and split:
                            nc.sync.dma_start(
                                out=yt[:, s * R * KM + c6 + 3 * nch:
                                       s * R * KM + c6 + 5 * nch],
                                in_=ys[:, c6 + 3 * nch:c6 + 5 * nch])
                    if split:
                        nc.sync.dma_start(
                            out=yt[:, s * R * KM + c6 + 5 * nch:
                                   s * R * KM + c6 + KM * nch],
                            in_=ys[:, c6 + 5 * nch:c6 + KM * nch])
                    else:
                        nc.sync.dma_start(
                            out=yt[:, s * R * KM + c6:
                                   s * R * KM + c6 + KM * nch],
                            in_=ys[:, c6:c6 + KM * nch])
                if s != S - 1:
                    nc.sync.dma_start(out=ytt[:, s * R:(s + 1) * R], in_=yst)
                fill(6, pad=False)
                del res[s]
    nc.compile()
    return nc


_programs = {}


def _get_program(S, R, mode):
    if (S, R, mode) not in _programs:
        _programs[(S, R, mode)] = _build_program(S, R, mode)
    return _programs[(S, R, mode)]


def _pack_weights(params, slot_clusters):
    S = len(slot_clusters)
    wpk = np.zeros((128, S * WSLOT), np.float32)
    bpk = np.zeros((128, S * BSLOT), np.float32)
    for s, c in enumerate(slot_clusters):
        wb, bb = s * WSLOT, s * BSLOT
        we0, we1, we2 = params["w_e0"][c], params["w_e1"][c], params["w_e2"][c]
        wd0, wd1, wd2 = params["w_d0"][c], params["w_d1"][c], params["w_d2"][c]
        for k in range(KM):
            wpk[:, wb + _E0M + k * 256: wb + _E0M + (k + 1) * 256] = \
                we0[128 * k:128 * (k + 1), :]
        wpk[0:TR, wb + _E0T: wb + _E0T + 256] = we0[128 * KM:, :]
        for k in range(2):
            wpk[:, wb + _E1 + 64 * k: wb + _E1 + 64 * (k + 1)] = \
                we1[128 * k:128 * (k + 1), :]
        wpk[0:64, wb + _E2: wb + _E2 + 16] = we2
        wpk[0:16, wb + _D0: wb + _D0 + 64] = wd0
        wpk[0:64, wb + _D1: wb + _D1 + 256] = wd1
        for k in range(2):
            wpk[:, wb + _D2 + 784 * k: wb + _D2 + 784 * k + 768] = \
                wd2[128 * k:128 * (k + 1), 0:768]
            wpk[:, wb + _D2 + 784 * k + 768: wb + _D2 + 784 * (k + 1)] = \
                wd2[128 * k:128 * (k + 1), 768:784]

        be0, be1, be2 = params["b_e0"][c], params["b_e1"][c], params["b_e2"][c]
        bd0, bd1, bd2 = params["b_d0"][c], params["b_d1"][c], params["b_d2"][c]
        bpk[0:128, bb + 0] = be0[0:128]
        bpk[0:128, bb + 1] = be0[128:256]
        bpk[0:64, bb + 2] = be1
        bpk[0:16, bb + 3] = be2
        bpk[0:64, bb + 4] = bd0
        bpk[0:128, bb + 5] = bd1[0:128]
        bpk[0:128, bb + 6] = bd1[128:256]
        for m in range(KM):
            bpk[0:128, bb + 7 + m] = bd2[128 * m:128 * (m + 1)]
        bpk[0:TR, bb + 13] = bd2[128 * KM:]
    return wpk, bpk


def _route(labels, mode):
    """Assign rows to (core, slot) blocks; returns config + per-slot rows.

    bf16: S=4 slots/core; R is the minimax of ceil(count_c / n_c) over a
    greedy slot assignment (rounded up to 32), so padding tracks the
    actual histogram.  f32r: static fallback configs."""
    counts = np.bincount(labels, minlength=K)
    if mode == "bf16":
        S = 4
        nslots = N_CORES * S
        import heapq
        act = [c for c in range(K) if counts[c] > 0]
        R = None
        if 0 < len(act) <= nslots:
            n = {c: 1 for c in act}
            h = [(-int(np.ceil(counts[c] / n[c])), c) for c in act]
            heapq.heapify(h)
            for _ in range(nslots - len(act)):
                _, c = heapq.heappop(h)
                n[c] += 1
                heapq.heappush(h, (-int(np.ceil(counts[c] / n[c])), c))
            R = max(int(np.ceil(counts[c] / n[c])) for c in act)
            R = (R + 31) // 32 * 32
            if R > 1536:  # SBUF guard; cannot happen for B=32768, K<=10
                R = None
    else:
        R = None
    if R is None:
        configs = _CONFIGS if mode == "bf16" else _CONFIGS[1:]
        for S, R in configs:
            need = int(np.sum((counts + R - 1) // R))
            if need <= N_CORES * S:
                break
    nslots = N_CORES * S
    order = np.argsort(labels, kind="stable")
    slot_cluster = np.zeros(nslots, np.int64)
    slot_rows = [np.empty(0, np.int64)] * nslots
    si = pos = 0
    for c in range(K):
        cnt = int(counts[c])
        rows_c = order[pos:pos + cnt]
        pos += cnt
        for off in range(0, cnt, R):
            slot_cluster[si] = c
            slot_rows[si] = rows_c[off:off + R]
            si += 1
    return S, R, slot_cluster, slot_rows


def _flatten_xcore(xcore_t, R, chunks):
    """[D, S*R] feature-major slab -> ([128, S*R*KM] main, [TR, S*R] tail)."""
    ncols = xcore_t.shape[1]
    S = ncols // R
    main = np.empty((128, ncols * KM), np.float32)
    tailr = np.empty((TR, ncols), np.float32)
    pos = cpos = 0
    for s in range(S):
        col = s * R
        for nch in chunks:
            blk = xcore_t[0:128 * KM, col:col + nch]     # [768, nch]
            blk = blk.reshape(KM, 128, nch).transpose(1, 0, 2)
            main[:, pos:pos + KM * nch] = blk.reshape(128, KM * nch)
            tailr[:, cpos:cpos + nch] = xcore_t[128 * KM:, col:col + nch]
            pos += KM * nch
            cpos += nch
            col += nch
    return main, tailr


def _unflatten_ycore(ymain, ytail, R, chunks):
    """([128, S*R*KM], [TR, S*R]) -> row-major [S*R, D]."""
    ncols = ymain.shape[1] // KM
    S = ncols // R
    out = np.empty((ncols, D), np.float32)
    pos = cpos = 0
    for s in range(S):
        col = s * R
        for nch in chunks:
            blk = ymain[:, pos:pos + KM * nch].reshape(128, KM, nch)
            out[col:col + nch, 0:128 * KM] = \
                blk.transpose(2, 1, 0).reshape(nch, 128 * KM)
            out[col:col + nch, 128 * KM:] = ytail[:, cpos:cpos + nch].T
            pos += KM * nch
            cpos += nch
            col += nch
    return out


def kernel_traced(inputs, trace=False, mode=None):
    if mode is None:
        mode = MODE
    x = np.ascontiguousarray(np.asarray(inputs["x"], dtype=np.float32))
    labels = np.asarray(inputs["kmeans_label"]).astype(np.int64).ravel()
    params = {k: np.asarray(v, dtype=np.float32)
              for k, v in inputs.items() if k not in ("x", "kmeans_label")}

    S, R, slot_cluster, slot_rows = _route(labels, mode)
    chunks = _chunks(R, mode)
    nc = _get_program(S, R, mode)

    in_maps = []
    for i in range(N_CORES):
        xcore = np.zeros((S * R, D), np.float32)
        for s in range(S):
            rows = slot_rows[i * S + s]
            if len(rows):
                xcore[s * R: s * R + len(rows)] = x[rows]
        wpk, bpk = _pack_weights(params, slot_cluster[i * S:(i + 1) * S])
        xmain, xtail = _flatten_xcore(np.ascontiguousarray(xcore.T), R, chunks)
        if mode == "bf16":
            import ml_dtypes
            xmain = xmain.astype(ml_dtypes.bfloat16)
            xtail = xtail.astype(ml_dtypes.bfloat16)
            wpk = wpk.astype(ml_dtypes.bfloat16)
        in_maps.append({"xt": xmain, "xtt": xtail, "wp": wpk, "bp": bpk})

    res = run_bass_kernel_spmd(nc, in_maps, core_ids=list(range(N_CORES)),
                               trace=trace)

    out = np.zeros_like(x)
    for i in range(N_CORES):
        ymain = np.asarray(res.results[i]["yt"]).astype(np.float32)
        ytail = np.asarray(res.results[i]["ytt"]).astype(np.float32)
        ytT = _unflatten_ycore(ymain, ytail, R, chunks)
        for s in range(S):
            rows = slot_rows[i * S + s]
            if len(rows):
                out[rows] = ytT[s * R: s * R + len(rows)]
    return out, res


def kernel(**inputs):
    out, _ = kernel_traced(inputs, trace=False)
    return out
